# revision 1
# baseline (speedup 1.0000x reference)
"""Trainium2 Bass kernel for nn_Attn_48137993453608.

Module: Y = X@W1.T+b1 -> split Q,K,V -> w = softmax((Q_h^T K_h)/sqrt(S))
        (attention over the DH=64 dim, contracting S) -> out = w @ V_h^T
        -> raw memory-order reshape [B,H,DH,S]->[B,S,D] -> @ W2.T + b2.

Sharding: 8 cores = 4 batch x 2 head-groups (8 heads each). Each core owns a
contiguous [1024, 1024] block of the output (rows i = 128*h + 2*d + (s>=1024)
for its heads), so no collectives are needed.

Key reassociation: the final projection contracts the attention output over
j = s mod 1024, and the attention output is linear in V, so

  F_un[c2, n] = sum_j OT_un[j, c2] W2T[j, n]
              = sum_e expw[e, c2] * G[e, n],
  G_{p,half}[e, n] = sum_j V[half*1024+j, p*128+e] * W2T[j, n].

G is softmax-independent, so nearly all of the output-projection FLOPs run
inside phase 1's dense matmul stream; only a single [128x128]x[128x512]
matmul per (pair, half, nh) remains after the softmax.

Per-core dataflow:
  1. Y[s, :1536] = Xb @ Wqkv.T + b     (Q | K | V columns, local heads)
     Q,K columns feed wT; V columns are stored per s-tile (f32r).
  2. wT_h[e, d] = sum_s K_h[s,e] Q_h[s,d]   (PSUM accum over all s)
  3. G pass (still phase 1 PE work): G = V^T-blocks @ W2T-blocks
  4. expwT = exp(wT / sqrt(S))        (no max-sub: |logits| <= ~6)
     block-diag per head pair; Z via ones-matmul; rZ = 1/Z
  5. F_un = expw2^T-matmul against G; F = F_un * rZ[c2] + b2
  6. scatter F rows to the output block: r = 256*p + 128*g + 2*d + half

Precision: the whole matmul stream is bf16 (uniform dtype also avoids PE
weight-mode switches, worth ~7us here) with fp32 PSUM accumulation and fp32
bias adds.  Logits are soft (|logit| <= ~6) and Z is computed from the same
bf16 exp values used in the F matmul, so the normalized softmax weights sum
to exactly 1 and most rounding cancels.  Measured ~156us HW exec,
~5.2e-3 scale-relative absmax error vs the fp32 oracle.
"""

import os
import sys

for _p in ("/opt/trn_rl_repo",):
    if _p not in sys.path and os.path.isdir(_p):
        sys.path.insert(0, _p)

import ml_dtypes
import numpy as np

import concourse.bass as bass
import concourse.bacc as bacc
import concourse.mybir as mybir
import concourse.tile as tile
from concourse.bass_utils import run_bass_kernel_spmd

B, S, D, H = 4, 2048, 1024, 16
DH = D // H          # 64
NH = 8               # heads per core
SCALE = 1.0 / float(np.sqrt(np.float32(S)))

F32 = mybir.dt.float32
F32R = mybir.dt.float32r
BF16 = mybir.dt.bfloat16

S_CHUNK = 256                 # s columns of X^T staged per iteration
N_SCHUNKS = S // S_CHUNK      # 8
ST_PER_CHUNK = S_CHUNK // 128 # 2


def build_nc():
    nc = bacc.Bacc("TRN2", target_bir_lowering=False, debug=False)

    xbt = nc.dram_tensor("xbt", [D, S], BF16, kind="ExternalInput")        # X[b].T
    wqkvt = nc.dram_tensor("wqkvt", [D, 1536], BF16, kind="ExternalInput") # [D, Q|K|V rows]
    bqkv = nc.dram_tensor("bqkv", [1, 1536], F32, kind="ExternalInput")
    w2t = nc.dram_tensor("w2t", [D, 1024], BF16, kind="ExternalInput")     # W2.T
    b2 = nc.dram_tensor("b2", [1, 1024], F32, kind="ExternalInput")
    out = nc.dram_tensor("out", [1024, 1024], F32, kind="ExternalOutput")

    xbt_v = xbt[:].rearrange("(kb p) s -> p kb s", p=128)      # [128, 8, 2048]
    # output rows r = 256*p + 128*g + 2*d + half
    out_v = out[:].rearrange("(p g d h) n -> p g d h n", p=4, g=2, d=64, h=2)

    with tile.TileContext(nc) as tc:
        with (
            tc.tile_pool(name="const", bufs=1) as const,
            tc.tile_pool(name="xin", bufs=3) as xin,
            tc.tile_pool(name="ywork", bufs=8) as ywork,
            tc.tile_pool(name="vstore", bufs=1) as vstore,
            tc.tile_pool(name="attn", bufs=1) as attn,
            tc.tile_pool(name="fout", bufs=4) as fout,
            tc.tile_pool(name="psacc", bufs=3, space="PSUM") as psacc,
            tc.tile_pool(name="pswt", bufs=1, space="PSUM") as pswt,
            tc.tile_pool(name="psg", bufs=2, space="PSUM") as psg,
            tc.tile_pool(name="psf", bufs=2, space="PSUM") as psf,
        ):
            # ---------------- phase-1 loads (w2/b2 deferred) --------------
            xbt_tiles = []
            xbt_sb0 = xin.tile([128, 8, S_CHUNK], BF16, tag="xbt")
            nc.sync.dma_start(out=xbt_sb0[:], in_=xbt_v[:, :, 0:S_CHUNK])
            xbt_tiles.append(xbt_sb0)

            # Per-kb loads keep >=3KB contiguous bursts; first QK matmul can
            # start after just the kb=0 block.
            wqkv_sb = const.tile([128, 8, 1536], BF16)
            for kb in range(8):
                nc.scalar.dma_start(out=wqkv_sb[:, kb, :],
                                    in_=wqkvt[kb * 128:(kb + 1) * 128, :])

            b_bc = const.tile([128, 1536], F32)
            nc.gpsimd.dma_start(out=b_bc[:], in_=bqkv[:].to_broadcast((128, 1536)))

            ones_sb = const.tile([128, 1], BF16)
            nc.vector.memset(ones_sb[:], 1.0)

            # V (natural [s, vrow] layout) persists until the G pass;
            # wT accumulates across all s.
            v_sb = vstore.tile([128, 16, 512], BF16)
            psum_wt = pswt.tile([128, 512], F32)        # [e2(g*64+e), pair*128+c2]

            # ---------------- phase 1: QKV projection + wT ----------------
            for sc in range(N_SCHUNKS):
                if sc < len(xbt_tiles):
                    xbt_sb = xbt_tiles[sc]
                else:
                    xbt_sb = xin.tile([128, 8, S_CHUNK], BF16, tag="xbt")
                    nc.sync.dma_start(
                        out=xbt_sb[:],
                        in_=xbt_v[:, :, sc * S_CHUNK:(sc + 1) * S_CHUNK],
                    )

                yqk_tiles = []
                for st in range(ST_PER_CHUNK):
                    s_lo = st * 128
                    t_glob = sc * ST_PER_CHUNK + st
                    yqk_sb = ywork.tile([128, 1024], BF16, tag="yqk")
                    yqk_tiles.append(yqk_sb)
                    for nh in range(3):
                        ps_y = psacc.tile([128, 512], F32, tag="acc")
                        for kb in range(8):
                            nc.tensor.matmul(
                                ps_y[:],
                                lhsT=xbt_sb[:, kb, s_lo:s_lo + 128],
                                rhs=wqkv_sb[:, kb, nh * 512:(nh + 1) * 512],
                                start=(kb == 0),
                                stop=(kb == 7),
                            )
                        if nh < 2:
                            nc.vector.tensor_tensor(
                                out=yqk_sb[:, nh * 512:(nh + 1) * 512],
                                in0=ps_y[:],
                                in1=b_bc[:, nh * 512:(nh + 1) * 512],
                                op=mybir.AluOpType.add,
                            )
                        else:
                            nc.vector.tensor_tensor(
                                out=v_sb[:, t_glob, :],
                                in0=ps_y[:],
                                in1=b_bc[:, 1024:1536],
                                op=mybir.AluOpType.add,
                            )

                # wT accumulation: lhsT=K_h slice, rhs=Q_h slice.  Traced
                # after the full chunk's QKV so the DVE evictions have time
                # to drain before PE needs them.
                for st in range(ST_PER_CHUNK):
                    yqk_sb = yqk_tiles[st]
                    for hl in range(NH):
                        p, g = hl // 2, hl % 2
                        nc.tensor.matmul(
                            psum_wt[g * 64:(g + 1) * 64,
                                    p * 128 + g * 64:p * 128 + (g + 1) * 64],
                            lhsT=yqk_sb[:, 512 + hl * 64:512 + (hl + 1) * 64],
                            rhs=yqk_sb[:, hl * 64:(hl + 1) * 64],
                            # start=True clears has_written for the WHOLE bank
                            # row of the written partitions -> only the first
                            # matmul per partition-half may set it.
                            start=(sc == 0 and st == 0 and hl < 2),
                            stop=(sc == N_SCHUNKS - 1 and st == ST_PER_CHUNK - 1),
                            skip_group_check=True,
                        )

            # ---------------- phase-1.5 weights + G pass ------------------
            w2_sb = const.tile([128, 8, 1024], BF16)
            for jb in range(8):
                nc.scalar.dma_start(out=w2_sb[:, jb, :],
                                    in_=w2t[jb * 128:(jb + 1) * 128, :])
            b2_bc = const.tile([128, 1024], F32)
            nc.gpsimd.dma_start(out=b2_bc[:], in_=b2[:].to_broadcast((128, 1024)))

            # G_{p,half}[e2, n] = sum_j V[half*1024+j, p*128+e2] W2T[j, n]
            # (softmax-independent -> dense PE work before the exp barrier)
            g_sb = attn.tile([128, 16, 512], BF16)      # [(e2), p*4+half*2+nh, n]
            for p in range(4):
                for half in range(2):
                    for nh in range(2):
                        ps_g = psg.tile([128, 512], F32, tag="psg")
                        for jb in range(8):
                            nc.tensor.matmul(
                                ps_g[:],
                                lhsT=v_sb[:, half * 8 + jb,
                                          p * 128:(p + 1) * 128],
                                rhs=w2_sb[:, jb, nh * 512:(nh + 1) * 512],
                                start=(jb == 0),
                                stop=(jb == 7),
                            )
                        nc.vector.tensor_copy(
                            g_sb[:, p * 4 + half * 2 + nh, :], ps_g[:])

            # ---------------- phase 2: exp, Z (overlaps the G pass) -------
            expw_sb = attn.tile([128, 4, 128], BF16)
            nc.vector.memset(expw_sb[:], 0.0)
            for hl in range(NH):
                p, g = hl // 2, hl % 2
                nc.scalar.activation(
                    out=expw_sb[g * 64:(g + 1) * 64, p, g * 64:(g + 1) * 64],
                    in_=psum_wt[g * 64:(g + 1) * 64,
                                p * 128 + g * 64:p * 128 + (g + 1) * 64],
                    func=mybir.ActivationFunctionType.Exp,
                    scale=SCALE,
                )
            ps_z = psf.tile([128, 4], F32, tag="psf")
            rz_sb = attn.tile([128, 4], F32)
            for p in range(4):
                nc.tensor.matmul(
                    ps_z[:, p:p + 1],
                    lhsT=expw_sb[:, p, :],
                    rhs=ones_sb[:],
                    start=(p == 0),
                    stop=(p == 3),
                    skip_group_check=True,
                )
            nc.vector.reciprocal(rz_sb[:], ps_z[:])

            # ---------------- phase 3: F = expw^T x G, store --------------
            for p in range(4):
                for half in range(2):
                    f_sb = fout.tile([128, 1024], F32, tag="f")
                    for nh in range(2):
                        # alternate the two phase-1 pools -> 5-deep rotation
                        idx = (p * 2 + half) * 2 + nh
                        if idx % 5 < 3:
                            ps_f = psacc.tile([128, 512], F32, tag="acc")
                        else:
                            ps_f = psf.tile([128, 512], F32, tag="psf")
                        nc.tensor.matmul(
                            ps_f[:],
                            lhsT=expw_sb[:, p, :],
                            rhs=g_sb[:, p * 4 + half * 2 + nh, :],
                        )
                        # F = psum * rZ (per partition) + b2
                        nc.vector.scalar_tensor_tensor(
                            out=f_sb[:, nh * 512:(nh + 1) * 512],
                            in0=ps_f[:],
                            scalar=rz_sb[:, p:p + 1],
                            in1=b2_bc[:, nh * 512:(nh + 1) * 512],
                            op0=mybir.AluOpType.mult,
                            op1=mybir.AluOpType.add,
                        )
                    # alternate HWDGE queues so the 8 store transfers overlap
                    eng = nc.sync if (p * 2 + half) % 2 == 0 else nc.scalar
                    eng.dma_start(out=out_v[p, :, :, half, :], in_=f_sb[:])

    nc.finalize()
    return nc


_NC_CACHE = None


def _get_nc():
    global _NC_CACHE
    if _NC_CACHE is None:
        _NC_CACHE = build_nc()
    return _NC_CACHE


def _shard_inputs(X, W1, b1, W2, b2):
    X = np.asarray(X, np.float32)
    W1 = np.asarray(W1, np.float32)
    b1 = np.asarray(b1, np.float32)
    W2 = np.asarray(W2, np.float32)
    b2 = np.asarray(b2, np.float32)

    w2t = np.ascontiguousarray(W2.T).astype(ml_dtypes.bfloat16)
    b2r = np.ascontiguousarray(b2.reshape(1, 1024))
    xbts = [np.ascontiguousarray(X[b].T).astype(ml_dtypes.bfloat16)
            for b in range(B)]

    per_hg = []
    for hg in range(2):
        heads = range(NH * hg, NH * hg + NH)
        rows = np.concatenate(
            [np.arange(h * DH, (h + 1) * DH) for h in heads]
            + [D + np.arange(h * DH, (h + 1) * DH) for h in heads]
            + [2 * D + np.arange(h * DH, (h + 1) * DH) for h in heads])
        wqkvt = np.ascontiguousarray(W1[rows].T).astype(ml_dtypes.bfloat16)
        bqkv = np.ascontiguousarray(b1[rows].reshape(1, 1536))
        per_hg.append((wqkvt, bqkv))

    in_maps = []
    for c in range(8):
        b, hg = c // 2, c % 2
        wqkvt, bqkv = per_hg[hg]
        in_maps.append({
            "xbt": xbts[b], "wqkvt": wqkvt, "bqkv": bqkv,
            "w2t": w2t, "b2": b2r,
        })
    return in_maps


def run(X, W1, b1, W2, b2, **run_kwargs):
    """Returns (full_output, BassKernelResults)."""
    nc = _get_nc()
    in_maps = _shard_inputs(X, W1, b1, W2, b2)
    res = run_bass_kernel_spmd(nc, in_maps, core_ids=list(range(8)), **run_kwargs)
    full = np.empty((B, S, D), np.float32)
    for c in range(8):
        b, hg = c // 2, c % 2
        full[b, hg * 1024:(hg + 1) * 1024, :] = res.results[c]["out"]
    return full, res


def kernel(X, W1, b1, W2, b2):
    return run(X, W1, b1, W2, b2)[0]



# revision 4
# speedup vs baseline: 1.0183x; 1.0183x over previous
"""Trainium2 Bass kernel for nn_Attn_48137993453608.

Module: Y = X@W1.T+b1 -> split Q,K,V -> w = softmax((Q_h^T K_h)/sqrt(S))
        (attention over the DH=64 dim, contracting S) -> out = w @ V_h^T
        -> raw memory-order reshape [B,H,DH,S]->[B,S,D] -> @ W2.T + b2.

Sharding: 8 cores = 4 batch x 2 head-groups (8 heads each). Each core owns a
contiguous [1024, 1024] block of the output (rows i = 128*h + 2*d + (s>=1024)
for its heads), so no collectives are needed.

Key reassociation: the final projection contracts the attention output over
j = s mod 1024, and the attention output is linear in V, so

  F_un[c2, n] = sum_j OT_un[j, c2] W2T[j, n]
              = sum_e expw[e, c2] * G[e, n],
  G_{p,half}[e, n] = sum_j V[half*1024+j, p*128+e] W2T[j, n].

G is softmax-independent, so nearly all of the output-projection FLOPs run
inside the dense matmul stream; only a single [128x128]x[128x512] matmul per
(pair, half, nh) remains after the softmax.

v2 layout/schedule (from trace analysis of the 157us baseline; PE stream was
already dense at 215ns/512-row matmul, the losses were a 17us DMA-bound
lead-in and a 9us serial tail):
  - X is host-packed per chunk so each [128, 8x256] chunk loads with 4KB
    per-partition descriptors in ONE dma (vs 512B descriptors at ~43B/ns).
    All 8 chunks are resident in SBUF (32KB/partition).
  - W1 is host-packed [p, nh, kb, c] and loaded nh-major (Q cols first),
    kb 0-3 / 4-7 split across two queues, so the first PSUM group's
    accumulation chain never outruns the weight stream.
  - wT (logit) matmuls are paired: one [128x128] matmul per head-pair per
    s-tile; the cross-head off-diagonal blocks are garbage that is never
    read (exp reads only the diagonal 64x64 blocks).
  - The G pass is interleaved with phase 3: after each (p,half) G group, one
    lagging F unit (2 matmuls + normalize + store) is emitted, so DVE work
    and output stores overlap the remaining G matmuls instead of trailing
    the kernel. exp/Z run on Scalar/PE inside the first two G groups.
  - Output tensor is bf16 (store bytes halved; host converts to f32).

Precision: the whole matmul stream is bf16 (uniform dtype also avoids PE
weight-mode switches) with fp32 PSUM accumulation and fp32 bias adds.
Logits are soft (|logit| <= ~6) so exp needs no max-subtraction; Z is
computed from the same bf16 exp values used in the F matmul, so the
normalized softmax weights sum to exactly 1 and most rounding cancels.
"""

import os
import sys

for _p in ("/opt/trn_rl_repo",):
    if _p not in sys.path and os.path.isdir(_p):
        sys.path.insert(0, _p)

import ml_dtypes
import numpy as np

import concourse.bass as bass
import concourse.bacc as bacc
import concourse.mybir as mybir
import concourse.tile as tile
from concourse.bass_utils import run_bass_kernel_spmd

B, S, D, H = 4, 2048, 1024, 16
DH = D // H          # 64
NH = 8               # heads per core
SCALE = 1.0 / float(np.sqrt(np.float32(S)))

F32 = mybir.dt.float32
BF16 = mybir.dt.bfloat16

S_CHUNK = 256                 # s columns of X^T per chunk
N_SCHUNKS = S // S_CHUNK      # 8
ST_PER_CHUNK = S_CHUNK // 128 # 2


def build_nc():
    nc = bacc.Bacc("TRN2", target_bir_lowering=False, debug=False)

    # X^T packed per chunk: [sc, p, kb, si] = X[b, sc*256+si, kb*128+p]
    xp = nc.dram_tensor("xp", [N_SCHUNKS, 128, 8, S_CHUNK], BF16,
                        kind="ExternalInput")
    # W1^T packed: [p, nh, kb, c] = W1[rows[nh*512+c], kb*128+p]
    wq = nc.dram_tensor("wq", [128, 3, 8, 512], BF16, kind="ExternalInput")
    bqkv = nc.dram_tensor("bqkv", [1, 1536], F32, kind="ExternalInput")
    # W2^T packed: [p, jb, n] = W2[n, jb*128+p]
    w2p = nc.dram_tensor("w2p", [128, 8, 1024], BF16, kind="ExternalInput")
    b2 = nc.dram_tensor("b2", [1, 1024], F32, kind="ExternalInput")
    out = nc.dram_tensor("out", [1024, 1024], BF16, kind="ExternalOutput")

    # output rows r = 256*p + 128*g + 2*d + half
    out_v = out[:].rearrange("(p g d h) n -> p g d h n", p=4, g=2, d=64, h=2)

    with tile.TileContext(nc) as tc:
        with (
            tc.tile_pool(name="const", bufs=1) as const,
            tc.tile_pool(name="xin", bufs=1) as xin,
            tc.tile_pool(name="ywork", bufs=8) as ywork,
            tc.tile_pool(name="vstore", bufs=1) as vstore,
            tc.tile_pool(name="attn", bufs=1) as attn,
            tc.tile_pool(name="fout", bufs=4) as fout,
            tc.tile_pool(name="psacc", bufs=3, space="PSUM") as psacc,
            tc.tile_pool(name="pswt", bufs=1, space="PSUM") as pswt,
            tc.tile_pool(name="psg", bufs=2, space="PSUM") as psg,
            tc.tile_pool(name="psf", bufs=2, space="PSUM") as psf,
        ):
            # ---------------- input loads ---------------------------------
            # sync queue: the 8 X chunks (4KB/partition descriptors each).
            x_sb = xin.tile([128, N_SCHUNKS, 8, S_CHUNK], BF16)
            for sc in range(N_SCHUNKS):
                nc.sync.dma_start(out=x_sb[:, sc, :, :], in_=xp[sc])

            # gpsimd queue head: b_bc (gates the first PSUM eviction, tiny).
            b_bc = const.tile([128, 1536], F32)
            nc.gpsimd.dma_start(out=b_bc[:], in_=bqkv[:].to_broadcast((128, 1536)))

            # scalar/gpsimd queues: W1 nh-major (Q first), kb halves split
            # across the two queues so Q's 8 kb blocks land in ~half the time.
            wqkv_sb = const.tile([128, 3, 8, 512], BF16)
            for nh in range(3):
                nc.scalar.dma_start(out=wqkv_sb[:, nh, 0:4, :],
                                    in_=wq[:, nh, 0:4, :])
                nc.gpsimd.dma_start(out=wqkv_sb[:, nh, 4:8, :],
                                    in_=wq[:, nh, 4:8, :])

            # W2 after W1 on the same two queues; needed only by the G pass.
            w2_sb = const.tile([128, 8, 1024], BF16)
            nc.scalar.dma_start(out=w2_sb[:, 0:4, :], in_=w2p[:, 0:4, :])
            nc.gpsimd.dma_start(out=w2_sb[:, 4:8, :], in_=w2p[:, 4:8, :])
            b2_bc = const.tile([128, 1024], F32)
            nc.gpsimd.dma_start(out=b2_bc[:], in_=b2[:].to_broadcast((128, 1024)))

            ones_sb = const.tile([128, 1], BF16)
            nc.vector.memset(ones_sb[:], 1.0)

            expw_sb = attn.tile([128, 4, 128], BF16)
            nc.vector.memset(expw_sb[:], 0.0)

            # V (natural [s, vrow] layout) persists until the G pass;
            # wT accumulates across all s.
            v_sb = vstore.tile([128, 16, 512], BF16)
            psum_wt = pswt.tile([128, 512], F32)   # [e2, pair*128 + c2]

            # ---------------- phase 1: QKV projection + wT ----------------
            for sc in range(N_SCHUNKS):
                yqk_tiles = []
                for st in range(ST_PER_CHUNK):
                    s_lo = st * 128
                    t_glob = sc * ST_PER_CHUNK + st
                    yqk_sb = ywork.tile([128, 1024], BF16, tag="yqk")
                    yqk_tiles.append(yqk_sb)
                    for nh in range(3):
                        ps_y = psacc.tile([128, 512], F32, tag="acc")
                        for kb in range(8):
                            nc.tensor.matmul(
                                ps_y[:],
                                lhsT=x_sb[:, sc, kb, s_lo:s_lo + 128],
                                rhs=wqkv_sb[:, nh, kb, :],
                                start=(kb == 0),
                                stop=(kb == 7),
                            )
                        if nh < 2:
                            nc.vector.tensor_tensor(
                                out=yqk_sb[:, nh * 512:(nh + 1) * 512],
                                in0=ps_y[:],
                                in1=b_bc[:, nh * 512:(nh + 1) * 512],
                                op=mybir.AluOpType.add,
                            )
                        else:
                            nc.vector.tensor_tensor(
                                out=v_sb[:, t_glob, :],
                                in0=ps_y[:],
                                in1=b_bc[:, 1024:1536],
                                op=mybir.AluOpType.add,
                            )

                # paired wT accumulation: one [128x128] matmul per head pair;
                # off-diagonal 64x64 blocks are cross-head garbage, never
                # read.  Traced after the full chunk's QKV so DVE evictions
                # have time to drain.
                for st in range(ST_PER_CHUNK):
                    yqk_sb = yqk_tiles[st]
                    for p in range(4):
                        nc.tensor.matmul(
                            psum_wt[:, p * 128:(p + 1) * 128],
                            lhsT=yqk_sb[:, 512 + p * 128:512 + (p + 1) * 128],
                            rhs=yqk_sb[:, p * 128:(p + 1) * 128],
                            # start=True clears has_written for the whole
                            # bank row -> only the very first matmul sets it.
                            start=(sc == 0 and st == 0 and p == 0),
                            stop=(sc == N_SCHUNKS - 1
                                  and st == ST_PER_CHUNK - 1 and p == 3),
                            skip_group_check=True,
                        )

            # ---------------- phase 2 helpers (emitted up-front) ----------
            # exp on Scalar overlaps the first G groups on PE.
            for hl in range(NH):
                p, g = hl // 2, hl % 2
                nc.scalar.activation(
                    out=expw_sb[g * 64:(g + 1) * 64, p, g * 64:(g + 1) * 64],
                    in_=psum_wt[g * 64:(g + 1) * 64,
                                p * 128 + g * 64:p * 128 + (g + 1) * 64],
                    func=mybir.ActivationFunctionType.Exp,
                    scale=SCALE,
                )

            g_sb = attn.tile([128, 16, 512], BF16)   # [(e2), p*4+half*2+nh, n]
            rz_sb = attn.tile([128, 4], F32)

            def g_group(p, half):
                for nh in range(2):
                    ps_g = psg.tile([128, 512], F32, tag="psg")
                    for jb in range(8):
                        nc.tensor.matmul(
                            ps_g[:],
                            lhsT=v_sb[:, half * 8 + jb, p * 128:(p + 1) * 128],
                            rhs=w2_sb[:, jb, nh * 512:(nh + 1) * 512],
                            start=(jb == 0),
                            stop=(jb == 7),
                        )
                    # eviction on Scalar keeps DVE free for the F units
                    # (GPSIMD cannot read PSUM)
                    nc.scalar.activation(
                        out=g_sb[:, p * 4 + half * 2 + nh, :],
                        in_=ps_g[:],
                        func=mybir.ActivationFunctionType.Copy,
                    )

            def z_block():
                ps_z = psf.tile([128, 4], F32, tag="psf")
                for p in range(4):
                    nc.tensor.matmul(
                        ps_z[:, p:p + 1],
                        lhsT=expw_sb[:, p, :],
                        rhs=ones_sb[:],
                        start=(p == 0),
                        stop=(p == 3),
                        skip_group_check=True,
                    )
                nc.vector.reciprocal(rz_sb[:], ps_z[:])

            f_idx = [0]

            def f_unit(p, half):
                f_sb = fout.tile([128, 1024], BF16, tag="f")
                for nh in range(2):
                    idx = f_idx[0]
                    f_idx[0] += 1
                    if idx % 5 < 3:
                        ps_f = psacc.tile([128, 512], F32, tag="acc")
                    else:
                        ps_f = psf.tile([128, 512], F32, tag="psf")
                    nc.tensor.matmul(
                        ps_f[:],
                        lhsT=expw_sb[:, p, :],
                        rhs=g_sb[:, p * 4 + half * 2 + nh, :],
                    )
                    # F = psum * rZ (per partition) + b2
                    nc.vector.scalar_tensor_tensor(
                        out=f_sb[:, nh * 512:(nh + 1) * 512],
                        in0=ps_f[:],
                        scalar=rz_sb[:, p:p + 1],
                        in1=b2_bc[:, nh * 512:(nh + 1) * 512],
                        op0=mybir.AluOpType.mult,
                        op1=mybir.AluOpType.add,
                    )
                eng = nc.sync if (p * 2 + half) % 2 == 0 else nc.scalar
                eng.dma_start(out=out_v[p, :, :, half, :], in_=f_sb[:])

            # ------------- phases 1.5/2/3 interleaved ---------------------
            # PE order: G00 G01 Z F00 G10 F01 G11 F10 ... G31 F30 F31
            units = [(p, half) for p in range(4) for half in range(2)]
            g_group(*units[0])
            g_group(*units[1])
            z_block()
            for k in range(2, 8):
                f_unit(*units[k - 2])
                g_group(*units[k])
            f_unit(*units[6])
            f_unit(*units[7])

    nc.finalize()
    return nc


_NC_CACHE = None


def _get_nc():
    global _NC_CACHE
    if _NC_CACHE is None:
        _NC_CACHE = build_nc()
    return _NC_CACHE


def _shard_inputs(X, W1, b1, W2, b2):
    X = np.asarray(X, np.float32)
    W1 = np.asarray(W1, np.float32)
    b1 = np.asarray(b1, np.float32)
    W2 = np.asarray(W2, np.float32)
    b2 = np.asarray(b2, np.float32)

    # W2^T packed [p, jb, n]
    w2t = W2.T  # [j, n]
    w2pk = np.ascontiguousarray(
        w2t.reshape(8, 128, 1024).transpose(1, 0, 2)).astype(ml_dtypes.bfloat16)
    b2r = np.ascontiguousarray(b2.reshape(1, 1024))

    # X chunks packed [sc, p, kb, si]
    xps = [np.ascontiguousarray(
               X[b].reshape(N_SCHUNKS, S_CHUNK, 8, 128).transpose(0, 3, 2, 1)
           ).astype(ml_dtypes.bfloat16)
           for b in range(B)]

    per_hg = []
    for hg in range(2):
        heads = range(NH * hg, NH * hg + NH)
        rows = np.concatenate(
            [np.arange(h * DH, (h + 1) * DH) for h in heads]
            + [D + np.arange(h * DH, (h + 1) * DH) for h in heads]
            + [2 * D + np.arange(h * DH, (h + 1) * DH) for h in heads])
        wqkvt = W1[rows].T  # [d, nh*512+c]
        wqpk = np.ascontiguousarray(
            wqkvt.reshape(8, 128, 3, 512).transpose(1, 2, 0, 3)
        ).astype(ml_dtypes.bfloat16)
        bqkv = np.ascontiguousarray(b1[rows].reshape(1, 1536))
        per_hg.append((wqpk, bqkv))

    in_maps = []
    for c in range(8):
        b, hg = c // 2, c % 2
        wqpk, bqkv = per_hg[hg]
        in_maps.append({
            "xp": xps[b], "wq": wqpk, "bqkv": bqkv,
            "w2p": w2pk, "b2": b2r,
        })
    return in_maps


def run(X, W1, b1, W2, b2, **run_kwargs):
    """Returns (full_output, BassKernelResults)."""
    nc = _get_nc()
    in_maps = _shard_inputs(X, W1, b1, W2, b2)
    res = run_bass_kernel_spmd(nc, in_maps, core_ids=list(range(8)), **run_kwargs)
    full = np.empty((B, S, D), np.float32)
    for c in range(8):
        b, hg = c // 2, c % 2
        full[b, hg * 1024:(hg + 1) * 1024, :] = res.results[c]["out"].astype(
            np.float32)
    return full, res


def kernel(X, W1, b1, W2, b2):
    return run(X, W1, b1, W2, b2)[0]


# revision 5
# speedup vs baseline: 1.0248x; 1.0064x over previous
"""Trainium2 Bass kernel for nn_Attn_48137993453608.

Module: Y = X@W1.T+b1 -> split Q,K,V -> w = softmax((Q_h^T K_h)/sqrt(S))
        (attention over the DH=64 dim, contracting S) -> out = w @ V_h^T
        -> raw memory-order reshape [B,H,DH,S]->[B,S,D] -> @ W2.T + b2.

Sharding: 8 cores = 4 batch x 2 head-groups (8 heads each). Each core owns a
contiguous [1024, 1024] block of the output (rows i = 128*h + 2*d + (s>=1024)
for its heads), so no collectives are needed.

Key reassociation: the final projection contracts the attention output over
j = s mod 1024, and the attention output is linear in V, so

  F_un[c2, n] = sum_j OT_un[j, c2] W2T[j, n]
              = sum_e expw[e, c2] * G[e, n],
  G_{p,half}[e, n] = sum_j V[half*1024+j, p*128+e] W2T[j, n].

G is softmax-independent, so nearly all of the output-projection FLOPs run
inside the dense matmul stream; only a single [128x128]x[128x512] matmul per
(pair, half, nh) remains after the softmax.  Additionally, since the
normalized softmax weights sum to exactly 1, b2 is folded into G's eviction
(F = (expw^T (G+b2)) * rZ), which moves the F normalization from DVE to the
Scalar engine and off the kernel tail.

v3 schedule (from trace analysis; PE stream is dense at 215ns/512-row
matmul, so the only levers are lead-in, dependency stalls and the tail):
  - Phase 1 runs nh-MAJOR: a Q pass over all 16 s-tiles, then a K pass,
    then a V pass (wT logit matmuls ride inside the V pass).  The kernel
    start therefore only needs X chunk 0 + the Q weights (1.5MB), and the
    K/V/W2 streams have 28..90us of slack instead of racing the PE.
  - X is host-packed so each [128, 8x256] chunk loads with 4KB
    per-partition descriptors; all 8 chunks are SBUF-resident.
  - DMA pieces are placed on the 3 queues (sync/scalar/gpsimd) in need
    order, sized so everything needed at T0 lands together.
  - wT matmuls are paired: one [128x128] matmul per head-pair per s-tile
    (cross-head off-diagonal blocks are garbage, never read).
  - G groups interleave with lag-1 F units; only the last F unit (2
    matmuls + scalar normalize + store) trails the last G matmul.
  - Output tensor is bf16 (store bytes halved; host converts to f32).

Precision: all-bf16 matmul stream (uniform dtype avoids PE weight-mode
switches) with fp32 PSUM accumulation and fp32 bias adds; logits are soft
(|logit| <= ~6) so exp needs no max-subtraction.
"""

import os
import sys

for _p in ("/opt/trn_rl_repo",):
    if _p not in sys.path and os.path.isdir(_p):
        sys.path.insert(0, _p)

import ml_dtypes
import numpy as np

import concourse.bass as bass
import concourse.bacc as bacc
import concourse.mybir as mybir
import concourse.tile as tile
from concourse.bass_utils import run_bass_kernel_spmd

B, S, D, H = 4, 2048, 1024, 16
DH = D // H          # 64
NH = 8               # heads per core
SCALE = 1.0 / float(np.sqrt(np.float32(S)))

F32 = mybir.dt.float32
BF16 = mybir.dt.bfloat16

S_CHUNK = 256
N_SCHUNKS = S // S_CHUNK      # 8
NT = 16                       # s-tiles of 128


def build_nc():
    nc = bacc.Bacc("TRN2", target_bir_lowering=False, debug=False)

    # X^T packed per chunk: [sc, p, kb, si] = X[b, sc*256+si, kb*128+p]
    xp = nc.dram_tensor("xp", [N_SCHUNKS, 128, 8, S_CHUNK], BF16,
                        kind="ExternalInput")
    # W1^T packed: [p, nh, kb, c] = W1[rows[nh*512+c], kb*128+p]
    wq = nc.dram_tensor("wq", [128, 3, 8, 512], BF16, kind="ExternalInput")
    bqkv = nc.dram_tensor("bqkv", [1, 1536], F32, kind="ExternalInput")
    # W2^T packed: [p, jb, n] = W2[n, jb*128+p]
    w2p = nc.dram_tensor("w2p", [128, 8, 1024], BF16, kind="ExternalInput")
    b2 = nc.dram_tensor("b2", [1, 1024], F32, kind="ExternalInput")
    out = nc.dram_tensor("out", [1024, 1024], BF16, kind="ExternalOutput")

    # output rows r = 256*p + 128*g + 2*d + half
    out_v = out[:].rearrange("(p g d h) n -> p g d h n", p=4, g=2, d=64, h=2)

    with tile.TileContext(nc) as tc:
        with (
            tc.tile_pool(name="const", bufs=1) as const,
            tc.tile_pool(name="xin", bufs=1) as xin,
            tc.tile_pool(name="ystore", bufs=1) as ystore,
            tc.tile_pool(name="attn", bufs=1) as attn,
            tc.tile_pool(name="fout", bufs=4) as fout,
            tc.tile_pool(name="psacc", bufs=3, space="PSUM") as psacc,
            tc.tile_pool(name="pswt", bufs=1, space="PSUM") as pswt,
            tc.tile_pool(name="psg", bufs=2, space="PSUM") as psg,
            tc.tile_pool(name="psf", bufs=2, space="PSUM") as psf,
        ):
            # ---------------- input loads (need-ordered) ------------------
            x_sb = xin.tile([128, N_SCHUNKS, 8, S_CHUNK], BF16)
            wqkv_sb = const.tile([128, 3, 8, 512], BF16)
            w2_sb = const.tile([128, 8, 1024], BF16)
            b_bc = const.tile([128, 1536], F32)
            b2_bc = const.tile([128, 1024], F32)

            def ld_x(eng, sc):
                eng.dma_start(out=x_sb[:, sc, :, :], in_=xp[sc])

            def ld_wq(eng, nh, k0, k1):
                eng.dma_start(out=wqkv_sb[:, nh, k0:k1, :],
                              in_=wq[:, nh, k0:k1, :])

            # gpsimd: b_bc, x0, Q[kb7], x1, K[kb0-3], V[kb0-3], w2[0-3], b2
            nc.gpsimd.dma_start(out=b_bc[:],
                                in_=bqkv[:].to_broadcast((128, 1536)))
            ld_x(nc.gpsimd, 0)
            ld_wq(nc.gpsimd, 0, 7, 8)
            ld_x(nc.gpsimd, 1)
            ld_wq(nc.gpsimd, 1, 0, 4)
            ld_wq(nc.gpsimd, 2, 0, 4)
            nc.gpsimd.dma_start(out=w2_sb[:, 0:4, :], in_=w2p[:, 0:4, :])
            nc.gpsimd.dma_start(out=b2_bc[:],
                                in_=b2[:].to_broadcast((128, 1024)))

            # sync: Q[kb0-4], x2..x5, (stores later)
            ld_wq(nc.sync, 0, 0, 5)
            for sc in (2, 3, 4, 5):
                ld_x(nc.sync, sc)

            # scalar: Q[kb5-6], x6, x7, K[kb4-7], V[kb4-7], w2[4-7]
            ld_wq(nc.scalar, 0, 5, 7)
            ld_x(nc.scalar, 6)
            ld_x(nc.scalar, 7)
            ld_wq(nc.scalar, 1, 4, 8)
            ld_wq(nc.scalar, 2, 4, 8)
            nc.scalar.dma_start(out=w2_sb[:, 4:8, :], in_=w2p[:, 4:8, :])

            ones_sb = const.tile([128, 1], BF16)
            nc.vector.memset(ones_sb[:], 1.0)
            expw_sb = attn.tile([128, 4, 128], BF16)
            nc.vector.memset(expw_sb[:], 0.0)

            # y_sb[:, nh, t, :]: Q / K / V rows for s-tile t
            y_sb = ystore.tile([128, 3, NT, 512], BF16)
            psum_wt = pswt.tile([128, 512], F32)   # [e2, pair*128 + c2]

            # ---------------- phase 1: nh-major QKV + wT ------------------
            for nh in range(3):
                for t in range(NT):
                    sc, st = t // 2, t % 2
                    ps_y = psacc.tile([128, 512], F32, tag="acc")
                    for kb in range(8):
                        nc.tensor.matmul(
                            ps_y[:],
                            lhsT=x_sb[:, sc, kb, st * 128:(st + 1) * 128],
                            rhs=wqkv_sb[:, nh, kb, :],
                            start=(kb == 0),
                            stop=(kb == 7),
                        )
                    nc.vector.tensor_tensor(
                        out=y_sb[:, nh, t, :],
                        in0=ps_y[:],
                        in1=b_bc[:, nh * 512:(nh + 1) * 512],
                        op=mybir.AluOpType.add,
                    )
                    if nh == 2:
                        # paired wT logit matmuls ride inside the V pass
                        for p in range(4):
                            nc.tensor.matmul(
                                psum_wt[:, p * 128:(p + 1) * 128],
                                lhsT=y_sb[:, 1, t, p * 128:(p + 1) * 128],
                                rhs=y_sb[:, 0, t, p * 128:(p + 1) * 128],
                                # start=True clears has_written for the whole
                                # bank row -> only the first matmul sets it.
                                start=(t == 0 and p == 0),
                                stop=(t == NT - 1 and p == 3),
                                skip_group_check=True,
                            )

            # ---------------- phase 2 helpers -----------------------------
            # exp on Scalar overlaps the first two G groups on PE.
            for hl in range(NH):
                p, g = hl // 2, hl % 2
                nc.scalar.activation(
                    out=expw_sb[g * 64:(g + 1) * 64, p, g * 64:(g + 1) * 64],
                    in_=psum_wt[g * 64:(g + 1) * 64,
                                p * 128 + g * 64:p * 128 + (g + 1) * 64],
                    func=mybir.ActivationFunctionType.Exp,
                    scale=SCALE,
                )

            g_sb = attn.tile([128, 16, 512], BF16)   # [(e2), p*4+half*2+nh, n]
            rz_sb = attn.tile([128, 4], F32)

            def g_group(p, half):
                for nh in range(2):
                    ps_g = psg.tile([128, 512], F32, tag="psg")
                    for jb in range(8):
                        nc.tensor.matmul(
                            ps_g[:],
                            lhsT=y_sb[:, 2, half * 8 + jb,
                                      p * 128:(p + 1) * 128],
                            rhs=w2_sb[:, jb, nh * 512:(nh + 1) * 512],
                            start=(jb == 0),
                            stop=(jb == 7),
                        )
                    # b2 folded here: softmax weights sum to exactly 1, so
                    # F = (expw^T (G+b2)) * rZ  ==  (expw^T G) * rZ + b2.
                    nc.vector.tensor_tensor(
                        out=g_sb[:, p * 4 + half * 2 + nh, :],
                        in0=ps_g[:],
                        in1=b2_bc[:, nh * 512:(nh + 1) * 512],
                        op=mybir.AluOpType.add,
                    )

            def z_block():
                ps_z = psf.tile([128, 4], F32, tag="psf")
                for p in range(4):
                    nc.tensor.matmul(
                        ps_z[:, p:p + 1],
                        lhsT=expw_sb[:, p, :],
                        rhs=ones_sb[:],
                        start=(p == 0),
                        stop=(p == 3),
                        skip_group_check=True,
                    )
                nc.vector.reciprocal(rz_sb[:], ps_z[:])

            f_idx = [0]

            def f_unit(p, half):
                f_sb = fout.tile([128, 1024], BF16, tag="f")
                for nh in range(2):
                    idx = f_idx[0]
                    f_idx[0] += 1
                    if idx % 5 < 3:
                        ps_f = psacc.tile([128, 512], F32, tag="acc")
                    else:
                        ps_f = psf.tile([128, 512], F32, tag="psf")
                    nc.tensor.matmul(
                        ps_f[:],
                        lhsT=expw_sb[:, p, :],
                        rhs=g_sb[:, p * 4 + half * 2 + nh, :],
                    )
                    # normalize on Scalar (DVE stays off the tail)
                    nc.scalar.activation(
                        out=f_sb[:, nh * 512:(nh + 1) * 512],
                        in_=ps_f[:],
                        func=mybir.ActivationFunctionType.Copy,
                        scale=rz_sb[:, p:p + 1],
                    )
                eng = nc.sync if (p * 2 + half) % 2 == 0 else nc.scalar
                eng.dma_start(out=out_v[p, :, :, half, :], in_=f_sb[:])

            # ------------- phases 1.5/2/3, lag-1 interleave ---------------
            units = [(p, half) for p in range(4) for half in range(2)]
            g_group(*units[0])
            g_group(*units[1])
            z_block()
            f_unit(*units[0])
            f_unit(*units[1])
            for k in range(2, 8):
                g_group(*units[k])
                f_unit(*units[k])

    nc.finalize()
    return nc


_NC_CACHE = None


def _get_nc():
    global _NC_CACHE
    if _NC_CACHE is None:
        _NC_CACHE = build_nc()
    return _NC_CACHE


def _shard_inputs(X, W1, b1, W2, b2):
    X = np.asarray(X, np.float32)
    W1 = np.asarray(W1, np.float32)
    b1 = np.asarray(b1, np.float32)
    W2 = np.asarray(W2, np.float32)
    b2 = np.asarray(b2, np.float32)

    w2t = W2.T  # [j, n]
    w2pk = np.ascontiguousarray(
        w2t.reshape(8, 128, 1024).transpose(1, 0, 2)).astype(ml_dtypes.bfloat16)
    b2r = np.ascontiguousarray(b2.reshape(1, 1024))

    xps = [np.ascontiguousarray(
               X[b].reshape(N_SCHUNKS, S_CHUNK, 8, 128).transpose(0, 3, 2, 1)
           ).astype(ml_dtypes.bfloat16)
           for b in range(B)]

    per_hg = []
    for hg in range(2):
        heads = range(NH * hg, NH * hg + NH)
        rows = np.concatenate(
            [np.arange(h * DH, (h + 1) * DH) for h in heads]
            + [D + np.arange(h * DH, (h + 1) * DH) for h in heads]
            + [2 * D + np.arange(h * DH, (h + 1) * DH) for h in heads])
        wqkvt = W1[rows].T  # [d, nh*512+c]
        wqpk = np.ascontiguousarray(
            wqkvt.reshape(8, 128, 3, 512).transpose(1, 2, 0, 3)
        ).astype(ml_dtypes.bfloat16)
        bqkv = np.ascontiguousarray(b1[rows].reshape(1, 1536))
        per_hg.append((wqpk, bqkv))

    in_maps = []
    for c in range(8):
        b, hg = c // 2, c % 2
        wqpk, bqkv = per_hg[hg]
        in_maps.append({
            "xp": xps[b], "wq": wqpk, "bqkv": bqkv,
            "w2p": w2pk, "b2": b2r,
        })
    return in_maps


def run(X, W1, b1, W2, b2, **run_kwargs):
    """Returns (full_output, BassKernelResults)."""
    nc = _get_nc()
    in_maps = _shard_inputs(X, W1, b1, W2, b2)
    res = run_bass_kernel_spmd(nc, in_maps, core_ids=list(range(8)), **run_kwargs)
    full = np.empty((B, S, D), np.float32)
    for c in range(8):
        b, hg = c // 2, c % 2
        full[b, hg * 1024:(hg + 1) * 1024, :] = res.results[c]["out"].astype(
            np.float32)
    return full, res


def kernel(X, W1, b1, W2, b2):
    return run(X, W1, b1, W2, b2)[0]


# revision 10
# speedup vs baseline: 1.0804x; 1.0543x over previous
"""Trainium2 Bass kernel for nn_Attn_48137993453608.

Module: Y = X@W1.T+b1 -> split Q,K,V -> w = softmax((Q_h^T K_h)/sqrt(S))
        (attention over the DH=64 dim, contracting S) -> out = w @ V_h^T
        -> raw memory-order reshape [B,H,DH,S]->[B,S,D] -> @ W2.T + b2.

Sharding: 8 cores = 4 batch x 2 head-groups (8 heads each). Each core owns a
contiguous [1024, 1024] block of the output (rows i = 128*h + 2*d + (s>=1024)
for its heads), so no collectives are needed.

Key reassociation: the final projection contracts the attention output over
j = s mod 1024, and the attention output is linear in V, so

  F_un[c2, n] = sum_j OT_un[j, c2] W2T[j, n]
              = sum_e expw[e, c2] * G[e, n],
  G_{p,half}[e, n] = sum_j V[half*1024+j, p*128+e] W2T[j, n].

G is softmax-independent, so nearly all of the output-projection FLOPs run
inside the dense matmul stream; only a single [128x128]x[128x512] matmul per
(pair, half, nh) remains after the softmax.  Additionally, since the
normalized softmax weights sum to exactly 1, b2 is folded into G's eviction
(F = (expw^T (G+b2)) * rZ), which moves the F normalization from DVE to the
Scalar engine and off the kernel tail.

v3 schedule (from trace analysis; PE stream is dense at 215ns/512-row
matmul, so the only levers are lead-in, dependency stalls and the tail):
  - Phase 1 runs nh-MAJOR: a Q pass over all 16 s-tiles, then a K pass,
    then a V pass (wT logit matmuls ride inside the V pass).  The kernel
    start therefore only needs X chunk 0 + the Q weights (1.5MB), and the
    K/V/W2 streams have 28..90us of slack instead of racing the PE.
  - X is host-packed so each [128, 8x256] chunk loads with 4KB
    per-partition descriptors; all 8 chunks are SBUF-resident.
  - DMA pieces are placed on the 3 queues (sync/scalar/gpsimd) in need
    order, sized so everything needed at T0 lands together.
  - wT matmuls are paired: one [128x128] matmul per head-pair per s-tile
    (cross-head off-diagonal blocks are garbage, never read).
  - G groups interleave with lag-1 F units; only the last F unit (2
    matmuls + scalar normalize + store) trails the last G matmul.
  - Output tensor is bf16 (store bytes halved; host converts to f32).

Precision: all-bf16 matmul stream (uniform dtype avoids PE weight-mode
switches) with fp32 PSUM accumulation and fp32 bias adds; logits are soft
(|logit| <= ~6) so exp needs no max-subtraction.
"""

import os
import sys

for _p in ("/opt/trn_rl_repo",):
    if _p not in sys.path and os.path.isdir(_p):
        sys.path.insert(0, _p)

import ml_dtypes
import numpy as np

import concourse.bass as bass
import concourse.bacc as bacc
import concourse.mybir as mybir
import concourse.tile as tile
from concourse.bass_utils import run_bass_kernel_spmd

B, S, D, H = 4, 2048, 1024, 16
DH = D // H          # 64
NH = 8               # heads per core
SCALE = 1.0 / float(np.sqrt(np.float32(S)))

F32 = mybir.dt.float32
BF16 = mybir.dt.bfloat16

S_CHUNK = 256
N_SCHUNKS = S // S_CHUNK      # 8
NT = 16                       # s-tiles of 128


def build_nc():
    nc = bacc.Bacc("TRN2", target_bir_lowering=False, debug=False)

    # X^T packed per chunk: [sc, p, kb, si] = X[b, sc*256+si, kb*128+p]
    xp = nc.dram_tensor("xp", [N_SCHUNKS, 128, 8, S_CHUNK], BF16,
                        kind="ExternalInput")
    # W1^T packed: [p, nh, kb, c] = W1[rows[nh*512+c], kb*128+p]
    wq = nc.dram_tensor("wq", [128, 3, 8, 512], BF16, kind="ExternalInput")
    bqkv = nc.dram_tensor("bqkv", [1, 1536], BF16, kind="ExternalInput")
    # W2^T packed: [p, jb, n] = W2[n, jb*128+p]
    w2p = nc.dram_tensor("w2p", [128, 8, 1024], BF16, kind="ExternalInput")
    b2 = nc.dram_tensor("b2", [1, 1024], BF16, kind="ExternalInput")
    out = nc.dram_tensor("out", [1024, 1024], BF16, kind="ExternalOutput")

    # output rows r = 256*p + 128*g + 2*d + half
    out_v = out[:].rearrange("(p g d h) n -> p g d h n", p=4, g=2, d=64, h=2)

    with tile.TileContext(nc) as tc:
        with (
            tc.tile_pool(name="const", bufs=1) as const,
            tc.tile_pool(name="xin", bufs=1) as xin,
            tc.tile_pool(name="ystore", bufs=1) as ystore,
            tc.tile_pool(name="attn", bufs=1) as attn,
            tc.tile_pool(name="fout", bufs=4) as fout,
            tc.tile_pool(name="psacc", bufs=3, space="PSUM") as psacc,
            tc.tile_pool(name="pswt", bufs=1, space="PSUM") as pswt,
            tc.tile_pool(name="psg", bufs=2, space="PSUM") as psg,
            tc.tile_pool(name="psf", bufs=2, space="PSUM") as psf,
        ):
            # ---------------- input loads (need-ordered) ------------------
            x_sb = xin.tile([128, N_SCHUNKS, 8, S_CHUNK], BF16)
            wqkv_sb = const.tile([128, 3, 8, 512], BF16)
            w2_sb = const.tile([128, 8, 1024], BF16)
            b_bc = const.tile([128, 1536], F32)
            b2_bc = const.tile([128, 1024], F32)

            b_row = const.tile([1, 1536], BF16)
            b2_row = const.tile([1, 1024], BF16)

            def ld_x(eng, sc):
                eng.dma_start(out=x_sb[:, sc, :, :], in_=xp[sc])

            def ld_wq(eng, nh, k0, k1):
                eng.dma_start(out=wqkv_sb[:, nh, k0:k1, :],
                              in_=wq[:, nh, k0:k1, :])

            # sync: x0, Q[kb0-2], x2..x5 (stores later)
            ld_x(nc.sync, 0)
            ld_wq(nc.sync, 0, 0, 3)
            for sc in (2, 3, 4, 5):
                ld_x(nc.sync, sc)

            # gpsimd: Q[kb3-5], x1, K[kb0-3], V[kb0-3], w2[0-3] (stores later)
            ld_wq(nc.gpsimd, 0, 3, 6)
            ld_x(nc.gpsimd, 1)
            ld_wq(nc.gpsimd, 1, 0, 4)
            ld_wq(nc.gpsimd, 2, 0, 4)
            nc.gpsimd.dma_start(out=w2_sb[:, 0:4, :], in_=w2p[:, 0:4, :])

            # scalar: bias rows (tiny), Q[kb6-7], x6, x7, K[4-7], V[4-7], w2[4-7]
            nc.scalar.dma_start(out=b_row[:], in_=bqkv[:])
            nc.scalar.dma_start(out=b2_row[:], in_=b2[:])
            ld_wq(nc.scalar, 0, 6, 8)
            ld_x(nc.scalar, 6)
            ld_x(nc.scalar, 7)
            ld_wq(nc.scalar, 1, 4, 8)
            ld_wq(nc.scalar, 2, 4, 8)
            nc.scalar.dma_start(out=w2_sb[:, 4:8, :], in_=w2p[:, 4:8, :])

            ones_sb = const.tile([128, 1], BF16)
            nc.vector.memset(ones_sb[:], 1.0)
            ones_row = const.tile([1, 128], BF16)
            nc.vector.memset(ones_row[:], 1.0)
            expw_sb = attn.tile([128, 4, 128], BF16)
            nc.vector.memset(expw_sb[:], 0.0)

            # broadcast biases on-chip: ones[1,128]^T @ b_row[1,512] per slice
            # (replaces 1.3MB of broadcast-DMA that clogged a queue head)
            for i in range(3):
                ps_b = psacc.tile([128, 512], F32, tag="acc")
                nc.tensor.matmul(ps_b[:], lhsT=ones_row[:],
                                 rhs=b_row[:, i * 512:(i + 1) * 512])
                nc.vector.tensor_copy(b_bc[:, i * 512:(i + 1) * 512], ps_b[:])
            for i in range(2):
                ps_b = psacc.tile([128, 512], F32, tag="acc")
                nc.tensor.matmul(ps_b[:], lhsT=ones_row[:],
                                 rhs=b2_row[:, i * 512:(i + 1) * 512])
                nc.vector.tensor_copy(b2_bc[:, i * 512:(i + 1) * 512], ps_b[:])

            # y_sb[:, nh, t, :]: Q / K / V rows for s-tile t
            y_sb = ystore.tile([128, 3, NT, 512], BF16)
            psum_wt = pswt.tile([128, 512], F32)   # [e2, pair*128 + c2]

            # ---------------- phase 1: nh-major QKV + wT ------------------
            for nh in range(3):
                for t in range(NT):
                    sc, st = t // 2, t % 2
                    ps_y = psacc.tile([128, 512], F32, tag="acc")
                    for kb in range(8):
                        nc.tensor.matmul(
                            ps_y[:],
                            lhsT=x_sb[:, sc, kb, st * 128:(st + 1) * 128],
                            rhs=wqkv_sb[:, nh, kb, :],
                            start=(kb == 0),
                            stop=(kb == 7),
                        )
                    nc.vector.tensor_tensor(
                        out=y_sb[:, nh, t, :],
                        in0=ps_y[:],
                        in1=b_bc[:, nh * 512:(nh + 1) * 512],
                        op=mybir.AluOpType.add,
                    )
                    if nh == 2:
                        # paired wT logit matmuls ride inside the V pass
                        for p in range(4):
                            nc.tensor.matmul(
                                psum_wt[:, p * 128:(p + 1) * 128],
                                lhsT=y_sb[:, 1, t, p * 128:(p + 1) * 128],
                                rhs=y_sb[:, 0, t, p * 128:(p + 1) * 128],
                                # start=True clears has_written for the whole
                                # bank row -> only the first matmul sets it.
                                start=(t == 0 and p == 0),
                                stop=(t == NT - 1 and p == 3),
                                skip_group_check=True,
                            )

            # ---------------- phase 2 helpers -----------------------------
            # exp on Scalar overlaps the first two G groups on PE.
            for hl in range(NH):
                p, g = hl // 2, hl % 2
                nc.scalar.activation(
                    out=expw_sb[g * 64:(g + 1) * 64, p, g * 64:(g + 1) * 64],
                    in_=psum_wt[g * 64:(g + 1) * 64,
                                p * 128 + g * 64:p * 128 + (g + 1) * 64],
                    func=mybir.ActivationFunctionType.Exp,
                    scale=SCALE,
                )

            g_sb = attn.tile([128, 16, 512], BF16)   # [(e2), p*4+half*2+nh, n]
            rz_sb = attn.tile([128, 4], F32)

            def g_group(p, half):
                for nh in range(2):
                    ps_g = psg.tile([128, 512], F32, tag="psg")
                    for jb in range(8):
                        nc.tensor.matmul(
                            ps_g[:],
                            lhsT=y_sb[:, 2, half * 8 + jb,
                                      p * 128:(p + 1) * 128],
                            rhs=w2_sb[:, jb, nh * 512:(nh + 1) * 512],
                            start=(jb == 0),
                            stop=(jb == 7),
                        )
                    # b2 folded here: softmax weights sum to exactly 1, so
                    # F = (expw^T (G+b2)) * rZ  ==  (expw^T G) * rZ + b2.
                    nc.vector.tensor_tensor(
                        out=g_sb[:, p * 4 + half * 2 + nh, :],
                        in0=ps_g[:],
                        in1=b2_bc[:, nh * 512:(nh + 1) * 512],
                        op=mybir.AluOpType.add,
                    )

            def z_block():
                ps_z = psf.tile([128, 4], F32, tag="psf")
                for p in range(4):
                    nc.tensor.matmul(
                        ps_z[:, p:p + 1],
                        lhsT=expw_sb[:, p, :],
                        rhs=ones_sb[:],
                        start=(p == 0),
                        stop=(p == 3),
                        skip_group_check=True,
                    )
                nc.vector.reciprocal(rz_sb[:], ps_z[:])

            f_idx = [0]

            def f_unit(p, half):
                f_sb = fout.tile([128, 1024], BF16, tag="f")
                for nh in range(2):
                    idx = f_idx[0]
                    f_idx[0] += 1
                    if idx % 5 < 3:
                        ps_f = psacc.tile([128, 512], F32, tag="acc")
                    else:
                        ps_f = psf.tile([128, 512], F32, tag="psf")
                    nc.tensor.matmul(
                        ps_f[:],
                        lhsT=expw_sb[:, p, :],
                        rhs=g_sb[:, p * 4 + half * 2 + nh, :],
                    )
                    # normalize on Scalar (DVE stays off the tail)
                    nc.scalar.activation(
                        out=f_sb[:, nh * 512:(nh + 1) * 512],
                        in_=ps_f[:],
                        func=mybir.ActivationFunctionType.Copy,
                        scale=rz_sb[:, p:p + 1],
                    )
                    # per-nh store on its own queue: the two halves of the
                    # last unit drain in parallel instead of serially
                    eng = nc.sync if nh == 0 else nc.gpsimd
                    eng.dma_start(
                        out=out_v[p, :, :, half, nh * 512:(nh + 1) * 512],
                        in_=f_sb[:, nh * 512:(nh + 1) * 512])

            # ------------- phases 1.5/2/3, lag-1 interleave ---------------
            units = [(p, half) for p in range(4) for half in range(2)]
            g_group(*units[0])
            g_group(*units[1])
            z_block()
            f_unit(*units[0])
            f_unit(*units[1])
            for k in range(2, 8):
                g_group(*units[k])
                f_unit(*units[k])

    nc.finalize()
    return nc


_NC_CACHE = None


def _get_nc():
    global _NC_CACHE
    if _NC_CACHE is None:
        _NC_CACHE = build_nc()
    return _NC_CACHE


def _shard_inputs(X, W1, b1, W2, b2):
    X = np.asarray(X, np.float32)
    W1 = np.asarray(W1, np.float32)
    b1 = np.asarray(b1, np.float32)
    W2 = np.asarray(W2, np.float32)
    b2 = np.asarray(b2, np.float32)

    w2t = W2.T  # [j, n]
    w2pk = np.ascontiguousarray(
        w2t.reshape(8, 128, 1024).transpose(1, 0, 2)).astype(ml_dtypes.bfloat16)
    b2r = np.ascontiguousarray(b2.reshape(1, 1024)).astype(ml_dtypes.bfloat16)

    xps = [np.ascontiguousarray(
               X[b].reshape(N_SCHUNKS, S_CHUNK, 8, 128).transpose(0, 3, 2, 1)
           ).astype(ml_dtypes.bfloat16)
           for b in range(B)]

    per_hg = []
    for hg in range(2):
        heads = range(NH * hg, NH * hg + NH)
        rows = np.concatenate(
            [np.arange(h * DH, (h + 1) * DH) for h in heads]
            + [D + np.arange(h * DH, (h + 1) * DH) for h in heads]
            + [2 * D + np.arange(h * DH, (h + 1) * DH) for h in heads])
        wqkvt = W1[rows].T  # [d, nh*512+c]
        wqpk = np.ascontiguousarray(
            wqkvt.reshape(8, 128, 3, 512).transpose(1, 2, 0, 3)
        ).astype(ml_dtypes.bfloat16)
        bqkv = np.ascontiguousarray(b1[rows].reshape(1, 1536)).astype(
            ml_dtypes.bfloat16)
        per_hg.append((wqpk, bqkv))

    in_maps = []
    for c in range(8):
        b, hg = c // 2, c % 2
        wqpk, bqkv = per_hg[hg]
        in_maps.append({
            "xp": xps[b], "wq": wqpk, "bqkv": bqkv,
            "w2p": w2pk, "b2": b2r,
        })
    return in_maps


def run(X, W1, b1, W2, b2, **run_kwargs):
    """Returns (full_output, BassKernelResults)."""
    nc = _get_nc()
    in_maps = _shard_inputs(X, W1, b1, W2, b2)
    res = run_bass_kernel_spmd(nc, in_maps, core_ids=list(range(8)), **run_kwargs)
    full = np.empty((B, S, D), np.float32)
    for c in range(8):
        b, hg = c // 2, c % 2
        full[b, hg * 1024:(hg + 1) * 1024, :] = res.results[c]["out"].astype(
            np.float32)
    return full, res


def kernel(X, W1, b1, W2, b2):
    return run(X, W1, b1, W2, b2)[0]
